# revision 2
# baseline (speedup 1.0000x reference)
"""Causal depthwise conv1d (B=8, C=1024, T=8192, K=4, dil=1) on 8 trn2 cores.

Sharding: batch-parallel — core j handles x[j] (1024, 8192), communication-free.

Per-core kernel (Bass/Tile):
  - channels -> 8 partition blocks of 128; time -> 4 chunks of 2048 (+3 halo)
  - conv as TensorE matmuls: lhsT = diag(w[:,k]) per (block, tap), rhs = the
    x tile shifted by k in the free dim; 4 taps accumulate in one PSUM bank.
    fp32r matmul mode (1 cyc/row at N=512) keeps PE under the DMA roofline.
  - PSUM -> SBUF eviction with fused per-channel bias on ScalarE.
  - HBM traffic is the roofline: 32 MiB in + 32 MiB out per core.
"""
import numpy as np

import concourse.bacc as bacc
import concourse.mybir as mybir
from concourse.tile import TileContext
from concourse import bass_utils

B, C, T, K = 8, 1024, 8192, 4
HALO = K - 1          # causal left pad
P = 128               # SBUF partitions
RBLK = C // P         # 8 channel blocks per core
CHUNK = 2048          # time chunk per inner iteration
NCHUNK = T // CHUNK   # 4
NGRP = CHUNK // 512   # psum groups per chunk

_cached = {}


def _build():
    nc = bacc.Bacc("TRN2", target_bir_lowering=False, debug=False)
    f32 = mybir.dt.float32
    f32r = mybir.dt.float32r

    x_d = nc.dram_tensor("x", [C, T], f32r, kind="ExternalInput")
    wd_d = nc.dram_tensor("wd", [P, RBLK * K * P], f32r, kind="ExternalInput")
    b_d = nc.dram_tensor("bv", [P, RBLK], f32, kind="ExternalInput")
    y_d = nc.dram_tensor("y", [C, T], f32, kind="ExternalOutput")

    with TileContext(nc) as tc:
        with (
            tc.tile_pool(name="const", bufs=1) as cpool,
            tc.tile_pool(name="io", bufs=4) as pool,
            tc.tile_pool(name="psum", bufs=8, space="PSUM") as psum_pool,
        ):
            wt = cpool.tile([P, RBLK * K * P], f32r)
            nc.sync.dma_start(out=wt, in_=wd_d.ap())
            bt = cpool.tile([P, RBLK], f32)
            nc.sync.dma_start(out=bt, in_=b_d.ap())

            for r in range(RBLK):
                rows = slice(r * P, (r + 1) * P)
                for i in range(NCHUNK):
                    xt = pool.tile([P, CHUNK + HALO], f32r, tag="xt")
                    if i == 0:
                        # memset doesn't support f32r; zero via uint32 view
                        nc.vector.memset(xt[:, 0:HALO].bitcast(mybir.dt.uint32), 0)
                        nc.sync.dma_start(out=xt[:, HALO:],
                                          in_=x_d.ap()[rows, 0:CHUNK])
                    else:
                        nc.sync.dma_start(
                            out=xt,
                            in_=x_d.ap()[rows, i * CHUNK - HALO:(i + 1) * CHUNK])

                    ot = pool.tile([P, CHUNK], f32, tag="ot")
                    for s in range(NGRP):
                        ps = psum_pool.tile([P, 512], f32)
                        for k in range(K):
                            nc.tensor.matmul(
                                ps,
                                wt[:, (r * K + k) * P:(r * K + k + 1) * P],
                                xt[:, s * 512 + k:s * 512 + k + 512],
                                start=(k == 0), stop=(k == K - 1))
                        nc.scalar.activation(
                            ot[:, s * 512:(s + 1) * 512], ps,
                            mybir.ActivationFunctionType.Identity,
                            bias=bt[:, r:r + 1], scale=1.0)
                    nc.sync.dma_start(
                        out=y_d.ap()[rows, i * CHUNK:(i + 1) * CHUNK], in_=ot)
    nc.compile()
    return nc


def _host_weights(w, b):
    # wd[p, (r*K+k)*P + m] = w[r*P+m, 0, k] if p == m else 0   (lhsT diagonals)
    wd = np.zeros((P, RBLK * K * P), dtype=np.float32)
    m = np.arange(P)
    for r in range(RBLK):
        for k in range(K):
            wd[m, (r * K + k) * P + m] = w[r * P + m, 0, k]
    bv = np.ascontiguousarray(b.reshape(RBLK, P).T).astype(np.float32)
    return wd, bv


def kernel(x, w, b):
    x = np.asarray(x, dtype=np.float32)
    w = np.asarray(w, dtype=np.float32)
    b = np.asarray(b, dtype=np.float32)

    if "nc" not in _cached:
        _cached["nc"] = _build()
    nc = _cached["nc"]

    wd, bv = _host_weights(w, b)
    in_maps = [
        {"x": np.ascontiguousarray(x[j]), "wd": wd, "bv": bv} for j in range(B)
    ]
    res = bass_utils.run_bass_kernel_spmd(nc, in_maps, core_ids=list(range(B)))
    return np.stack([r["y"] for r in res.results], axis=0)


# revision 3
# speedup vs baseline: 1.1837x; 1.1837x over previous
"""Causal depthwise conv1d (B=8, C=1024, T=8192, K=4, dil=1) on 8 trn2 cores.

Sharding: batch-parallel — core j handles x[j] (1024, 8192), communication-free.

Per-core kernel (Bass/Tile):
  - channels -> 8 partition blocks of 128; time -> 4 chunks of 2048 (+3 halo)
  - work split per 512-col psum group to keep every engine under the DMA
    roofline (~1.3us per group):
      PE:  taps 1..3 as fp32r matmuls with lhsT = diag(w[:,k]), rhs = the x
           tile shifted by k in the free dim, accumulated in one PSUM bank
      ACT: tap 0 fused with bias: tmp = x0 * w0 + bias (per-partition
           scale/bias APs)
      DVE: out = tmp + psum (tensor_tensor add), evicting PSUM
  - HBM traffic is the roofline: 32 MiB in + 32 MiB out per core.
"""
import numpy as np

import concourse.bacc as bacc
import concourse.mybir as mybir
from concourse.tile import TileContext
from concourse import bass_utils

B, C, T, K = 8, 1024, 8192, 4
HALO = K - 1          # causal left pad
P = 128               # SBUF partitions
RBLK = C // P         # 8 channel blocks per core
CHUNK = 2048          # time chunk per inner iteration
NCHUNK = T // CHUNK   # 4
NGRP = CHUNK // 512   # psum groups per chunk
NPE = K - 1           # taps done on PE (1..3); tap 0 rides the ACT pass

_cached = {}


def _build():
    nc = bacc.Bacc("TRN2", target_bir_lowering=False, debug=False)
    f32 = mybir.dt.float32
    f32r = mybir.dt.float32r

    x_d = nc.dram_tensor("x", [C, T], f32r, kind="ExternalInput")
    wd_d = nc.dram_tensor("wd", [P, RBLK * NPE * P], f32r, kind="ExternalInput")
    w0_d = nc.dram_tensor("w0", [P, RBLK], f32, kind="ExternalInput")
    b_d = nc.dram_tensor("bv", [P, RBLK], f32, kind="ExternalInput")
    y_d = nc.dram_tensor("y", [C, T], f32, kind="ExternalOutput")

    with TileContext(nc) as tc:
        with (
            tc.tile_pool(name="const", bufs=1) as cpool,
            tc.tile_pool(name="io", bufs=4) as pool,
            tc.tile_pool(name="tmp", bufs=8) as tpool,
            tc.tile_pool(name="psum", bufs=8, space="PSUM") as psum_pool,
        ):
            wt = cpool.tile([P, RBLK * NPE * P], f32r)
            nc.sync.dma_start(out=wt, in_=wd_d.ap())
            w0t = cpool.tile([P, RBLK], f32)
            nc.sync.dma_start(out=w0t, in_=w0_d.ap())
            bt = cpool.tile([P, RBLK], f32)
            nc.sync.dma_start(out=bt, in_=b_d.ap())

            for r in range(RBLK):
                rows = slice(r * P, (r + 1) * P)
                for i in range(NCHUNK):
                    xt = pool.tile([P, CHUNK + HALO], f32r, tag="xt")
                    if i == 0:
                        # memset doesn't support f32r; zero via uint32 view
                        nc.vector.memset(xt[:, 0:HALO].bitcast(mybir.dt.uint32), 0)
                        nc.sync.dma_start(out=xt[:, HALO:],
                                          in_=x_d.ap()[rows, 0:CHUNK])
                    else:
                        nc.sync.dma_start(
                            out=xt,
                            in_=x_d.ap()[rows, i * CHUNK - HALO:(i + 1) * CHUNK])
                    xf = xt.bitcast(f32)

                    ot = pool.tile([P, CHUNK], f32, tag="ot")
                    for s in range(NGRP):
                        ps = psum_pool.tile([P, 512], f32)
                        for k in range(1, K):
                            nc.tensor.matmul(
                                ps,
                                wt[:, (r * NPE + k - 1) * P:(r * NPE + k) * P],
                                xt[:, s * 512 + k:s * 512 + k + 512],
                                start=(k == 1), stop=(k == K - 1))
                        tmp = tpool.tile([P, 512], f32, tag="tmp")
                        nc.scalar.activation(
                            tmp, xf[:, s * 512:s * 512 + 512],
                            mybir.ActivationFunctionType.Identity,
                            bias=bt[:, r:r + 1], scale=w0t[:, r:r + 1])
                        nc.vector.tensor_add(
                            out=ot[:, s * 512:(s + 1) * 512], in0=tmp, in1=ps)
                    nc.sync.dma_start(
                        out=y_d.ap()[rows, i * CHUNK:(i + 1) * CHUNK], in_=ot)
    nc.compile()
    return nc


def _host_weights(w, b):
    # wd[p, (r*NPE+k-1)*P + m] = w[r*P+m, 0, k] if p == m else 0 (lhsT diags,
    # taps 1..K-1); tap 0 is applied by the ACT pass via w0.
    wd = np.zeros((P, RBLK * NPE * P), dtype=np.float32)
    m = np.arange(P)
    for r in range(RBLK):
        for k in range(1, K):
            wd[m, (r * NPE + k - 1) * P + m] = w[r * P + m, 0, k]
    w0 = np.ascontiguousarray(w[:, 0, 0].reshape(RBLK, P).T).astype(np.float32)
    bv = np.ascontiguousarray(b.reshape(RBLK, P).T).astype(np.float32)
    return wd, w0, bv


def kernel(x, w, b):
    x = np.asarray(x, dtype=np.float32)
    w = np.asarray(w, dtype=np.float32)
    b = np.asarray(b, dtype=np.float32)

    if "nc" not in _cached:
        _cached["nc"] = _build()
    nc = _cached["nc"]

    wd, w0, bv = _host_weights(w, b)
    in_maps = [
        {"x": np.ascontiguousarray(x[j]), "wd": wd, "w0": w0, "bv": bv}
        for j in range(B)
    ]
    res = bass_utils.run_bass_kernel_spmd(nc, in_maps, core_ids=list(range(B)))
    return np.stack([r["y"] for r in res.results], axis=0)
